# revision 10
# baseline (speedup 1.0000x reference)
"""Trainium2 Bass kernel for the dual-branch (dep/seq) transformer encoder.

Math notes (exact algebraic simplifications of the reference, not approximations):
- In the reference, `dep_out`/dep-FFN are dead code: only `dep_attn` (the dep
  branch's softmax probabilities) feeds forward into the seq branch scores.
- `hm = (help_scores > -1e8)` is always 1.0 since softmax output is in [0,1],
  so mixed scores = 0.5*dep_attn + 0.5*masked_seq_scores.
- Softmax is computed without max-subtraction (scores are O(1); masked entries
  reach exp() as exp(s)*mask01 with mask01 in {0,1}, which matches the
  reference's exp(-1e9 - max) == 0 exactly in fp32).

Layout: all activations are kept transposed ([feature, token]) so every linear
layer consumes the previous output directly as the stationary/moving matmul
operand and no on-chip transposes are needed. LayerNorm statistics and softmax
denominators (partition-dim reductions) are computed with all-ones matmuls on
the tensor engine, which also replicates them across partitions for free.

Sharding: data-parallel over batch; each of the 8 cores handles 8 batches.
"""

import numpy as np
import ml_dtypes
from contextlib import ExitStack

import concourse.bacc as bacc
import concourse.bass as bass
import concourse.tile as tile
from concourse import mybir

F32 = mybir.dt.float32
F32R = mybir.dt.float32r
BF16 = mybir.dt.bfloat16
AF = mybir.ActivationFunctionType

NCORES = 8
B, S, D = 64, 256, 512
H, DK = 8, 64
FF, L = 2048, 2
BC = B // NCORES          # batches per core
T = BC * S                # tokens per core
DC = D // 128             # feature chunks
FC = FF // 128            # ff chunks

_CACHE = {}


def _build_program(debug=False):
    nc = bacc.Bacc("TRN2", target_bir_lowering=False)

    def din(name, shape, dt=F32R):
        return nc.dram_tensor(name, shape, dt, kind="ExternalInput")

    xTd = din("xT", [D, T])
    m01d = din("m01_dep", [BC, 2, 128, S], BF16)
    m01s = din("m01_seq", [BC, 2, 128, S], BF16)
    constsd = din("consts", [128, 320])
    wd = {}
    for l in range(L):
        for nm, shape in [
            ("wq_d", [D, D]), ("wk_d", [D, D]),
            ("wq_s", [D, D]), ("wk_s", [D, D]), ("wv_s", [D, D]), ("wo_s", [D, D]),
            ("w1", [D, FF]), ("w2", [FF, D]),
        ]:
            wd[f"{nm}{l}"] = din(f"{nm}{l}", shape)
        for nm in ["bq_d", "bk_d", "bq_s", "bk_s", "bo_s", "b2", "g1", "be1", "g2", "be2"]:
            wd[f"{nm}{l}"] = din(f"{nm}{l}", [DC, 128], F32)
        wd[f"b1{l}"] = din(f"b1{l}", [FC, 128], F32)
        wd[f"bvrep{l}"] = din(f"bvrep{l}", [128, D], F32)
    outd = nc.dram_tensor("outT", [D, T], F32, kind="ExternalOutput")
    outd_r = outd.rearrange("(c p) t -> p c t", p=128)
    dbg = {}
    if debug:
        for nm, shape, dt in [
            ("dbg_qd", [128, DC, 256], BF16), ("dbg_kd", [128, DC, 256], BF16),
            ("dbg_vs", [128, 4, D], BF16), ("dbg_ed", [128, 2, 256], BF16),
            ("dbg_rd05", [128, 256], F32), ("dbg_es", [128, 2, 256], BF16),
            ("dbg_ctx", [128, DC, 256], F32), ("dbg_z", [128, DC, 256], F32),
            ("dbg_y1", [128, DC, 256], F32),
            ("dbg_cnum", [128, 256], F32), ("dbg_rf2", [128, 256], F32),
            ("dbg_den", [128, 256], F32), ("dbg_qdfull", [128, DC, 512], BF16),
            ("dbg_xt", [128, DC, 512], F32), ("dbg_vps", [128, 512], F32),
        ]:
            dbg[nm] = nc.dram_tensor(nm, shape, dt, kind="ExternalOutput")

    def wrearr(name):
        return wd[name].rearrange("(c p) o -> p c o", p=128)

    def brearr(name):
        return wd[name].rearrange("c p -> p c")

    with tile.TileContext(nc) as tc, ExitStack() as ctx:
        const = ctx.enter_context(tc.tile_pool(name="const", bufs=1))
        biasp = ctx.enter_context(tc.tile_pool(name="bias", bufs=1))
        xt_pool = ctx.enter_context(tc.tile_pool(name="xtp", bufs=1))
        y1t_pool = ctx.enter_context(tc.tile_pool(name="y1tp", bufs=1))

        consts = const.tile([128, 320], F32R, tag="consts")
        nc.sync.dma_start(out=consts[:], in_=constsd[:])
        ones2 = consts[:, 0:128]     # value 2.0   (dep denominator, folds the 0.5)
        ones1 = consts[:, 128:192]   # value 1.0   (seq denominator, M=64)
        onesLN = consts[:, 192:320]  # value 1/512 (layernorm stats)
        eps_t = const.tile([128, 1], F32, tag="eps")
        nc.vector.memset(eps_t[:], 1e-5)
        ones2b = const.tile([128, 128], BF16, tag="ones2b")
        nc.vector.tensor_copy(ones2b[:], ones2)
        ones1b = const.tile([128, 64], BF16, tag="ones1b")
        nc.vector.tensor_copy(ones1b[:], ones1)

        xt = xt_pool.tile([128, DC, T], F32R, tag="xt")
        nc.sync.dma_start(out=xt[:], in_=xTd.rearrange("(c p) t -> p c t", p=128))

        for l in range(L):
            # ---------------- biases for this layer ----------------
            bqd = biasp.tile([128, DC], F32, tag="bqd")
            nc.sync.dma_start(out=bqd[:], in_=brearr(f"bq_d{l}"))
            bkd = biasp.tile([128, DC], F32, tag="bkd")
            nc.sync.dma_start(out=bkd[:], in_=brearr(f"bk_d{l}"))
            bqs = biasp.tile([128, DC], F32, tag="bqs")
            nc.sync.dma_start(out=bqs[:], in_=brearr(f"bq_s{l}"))
            bks = biasp.tile([128, DC], F32, tag="bks")
            nc.sync.dma_start(out=bks[:], in_=brearr(f"bk_s{l}"))
            bos = biasp.tile([128, DC], F32, tag="bos")
            nc.sync.dma_start(out=bos[:], in_=brearr(f"bo_s{l}"))
            b2t = biasp.tile([128, DC], F32, tag="b2t")
            nc.sync.dma_start(out=b2t[:], in_=brearr(f"b2{l}"))
            g1t = biasp.tile([128, DC], F32, tag="g1t")
            nc.sync.dma_start(out=g1t[:], in_=brearr(f"g1{l}"))
            be1t = biasp.tile([128, DC], F32, tag="be1t")
            nc.sync.dma_start(out=be1t[:], in_=brearr(f"be1{l}"))
            g2t = biasp.tile([128, DC], F32, tag="g2t")
            nc.sync.dma_start(out=g2t[:], in_=brearr(f"g2{l}"))
            be2t = biasp.tile([128, DC], F32, tag="be2t")
            nc.sync.dma_start(out=be2t[:], in_=brearr(f"be2{l}"))
            b1t = biasp.tile([128, FC], F32, tag="b1t")
            nc.sync.dma_start(out=b1t[:], in_=wd[f"b1{l}"].rearrange("c p -> p c"))
            bvr = biasp.tile([128, D], F32, tag="bvr")
            nc.sync.dma_start(out=bvr[:], in_=wd[f"bvrep{l}"][:])

            y1t = y1t_pool.tile([128, DC, T], F32R, tag="y1t")

            # ================= attention phase =================
            with ExitStack() as actx:
                wat = actx.enter_context(tc.tile_pool(name=f"wat{l}", bufs=1))
                qkp = actx.enter_context(tc.tile_pool(name=f"qk{l}", bufs=1))
                vsp = actx.enter_context(tc.tile_pool(name=f"vs{l}", bufs=1))
                mp = actx.enter_context(tc.tile_pool(name=f"mp{l}", bufs=2))
                ep = actx.enter_context(tc.tile_pool(name=f"ep{l}", bufs=2))
                sp = actx.enter_context(tc.tile_pool(name=f"sp{l}", bufs=1))
                cp = actx.enter_context(tc.tile_pool(name=f"cp{l}", bufs=1))
                zp = actx.enter_context(tc.tile_pool(name=f"zp{l}", bufs=1))
                psb = actx.enter_context(tc.tile_pool(name=f"psb{l}", bufs=2, space="PSUM"))
                pssc = actx.enter_context(tc.tile_pool(name=f"pssc{l}", bufs=2, space="PSUM"))
                psacc = actx.enter_context(tc.tile_pool(name=f"psacc{l}", bufs=2, space="PSUM"))
                psrc = actx.enter_context(tc.tile_pool(name=f"psrc{l}", bufs=1, space="PSUM"))
                psmm = actx.enter_context(tc.tile_pool(name=f"psmm{l}", bufs=1, space="PSUM"))

                wqd = wat.tile([128, DC, D], F32R, tag="wqd")
                nc.sync.dma_start(out=wqd[:], in_=wrearr(f"wq_d{l}"))
                wkd = wat.tile([128, DC, D], F32R, tag="wkd")
                nc.sync.dma_start(out=wkd[:], in_=wrearr(f"wk_d{l}"))
                wqs = wat.tile([128, DC, D], F32R, tag="wqs")
                nc.sync.dma_start(out=wqs[:], in_=wrearr(f"wq_s{l}"))
                wks = wat.tile([128, DC, D], F32R, tag="wks")
                nc.sync.dma_start(out=wks[:], in_=wrearr(f"wk_s{l}"))
                wvs = wat.tile([128, DC, D], F32R, tag="wvs")
                nc.sync.dma_start(out=wvs[:], in_=wrearr(f"wv_s{l}"))
                wos = wat.tile([128, DC, D], F32R, tag="wos")
                nc.sync.dma_start(out=wos[:], in_=wrearr(f"wo_s{l}"))

                for g in range(4):  # groups of 2 batches (512 tokens)
                    ts0 = g * 512
                    qd = qkp.tile([128, DC, 512], BF16, tag="qd")
                    kd = qkp.tile([128, DC, 512], BF16, tag="kd")
                    qs = qkp.tile([128, DC, 512], BF16, tag="qs")
                    ks = qkp.tile([128, DC, 512], BF16, tag="ks")
                    vsg = vsp.tile([128, 4, D], BF16, tag="vs")
                    for w, bias, dest, scale in [
                        (wqd, bqd, qd, 0.125),
                        (wkd, bkd, kd, 1.0),
                        (wqs, bqs, qs, 0.0625),
                        (wks, bks, ks, 1.0),
                    ]:
                        for dc in range(DC):
                            ps = psb.tile([128, 512], F32, tag="psb")
                            for kc in range(DC):
                                nc.tensor.matmul(
                                    ps[:], w[:, kc, dc * 128:(dc + 1) * 128],
                                    xt[:, kc, ts0:ts0 + 512],
                                    start=(kc == 0), stop=(kc == DC - 1),
                                )
                            nc.scalar.activation(dest[:, dc, :], ps[:], AF.Identity,
                                                 bias=bias[:, dc:dc + 1], scale=scale)
                    for tti in range(4):  # V in [token, feature] layout
                        ps = psb.tile([128, 512], F32, tag="psb")
                        for kc in range(DC):
                            nc.tensor.matmul(
                                ps[:], xt[:, kc, ts0 + tti * 128:ts0 + (tti + 1) * 128],
                                wvs[:, kc, :],
                                start=(kc == 0), stop=(kc == DC - 1),
                            )
                        nc.vector.tensor_add(vsg[:, tti, :], ps[:], bvr[:])
                        if debug and g == 0 and tti == 0:
                            vps_t = sp.tile([128, 512], F32, tag="vps")
                            nc.scalar.activation(vps_t[:], ps[:], AF.Copy)
                            nc.sync.dma_start(out=dbg["dbg_vps"][:], in_=vps_t[:])

                    if debug and g == 0:
                        nc.sync.dma_start(out=dbg["dbg_qdfull"][:], in_=qd[:])
                        nc.sync.dma_start(out=dbg["dbg_xt"][:], in_=xt[:, :, 0:512].bitcast(F32))
                        nc.sync.dma_start(out=dbg["dbg_qd"][:], in_=qd[:, :, 0:256])
                        nc.sync.dma_start(out=dbg["dbg_kd"][:], in_=kd[:, :, 0:256])
                        nc.sync.dma_start(out=dbg["dbg_vs"][:], in_=vsg[:])
                    for lb in range(2):
                        b = g * 2 + lb
                        q0 = lb * 256
                        md = mp.tile([128, 2, S], BF16, tag="md")
                        nc.sync.dma_start(out=md[:], in_=m01d[b].rearrange("c p q -> p c q"))
                        ms = mp.tile([128, 2, S], BF16, tag="ms")
                        nc.sync.dma_start(out=ms[:], in_=m01s[b].rearrange("c p q -> p c q"))
                        ctxT = cp.tile([128, DC, 256], F32R, tag="ctx")
                        for hp in range(4):  # head pairs; pair hp = chunk hp, heads at rows 0/64
                            ps_rc = psrc.tile([128, 512], F32, tag="psrc")
                            ps_rf = ps_rc[:, 0:256]
                            ps_ctx = ps_rc[:, 256:512]
                            for hh in range(2):
                                h = hp * 2 + hh
                                hr = hh * 64
                                ed = ep.tile([128, 2, 256], BF16, tag="ed")
                                dah = ep.tile([128, 2, 256], F32, tag="dah")
                                es = ep.tile([128, 2, 256], BF16, tag="es")
                                ps_dd = psacc.tile([128, 256], F32, tag="psacc")
                                for kc in range(2):
                                    ps_sc = pssc.tile([128, 256], F32, tag="pssc")
                                    nc.tensor.matmul(
                                        ps_sc[:],
                                        kd[hr:hr + 64, hp, q0 + kc * 128:q0 + (kc + 1) * 128],
                                        qd[hr:hr + 64, hp, q0:q0 + 256],
                                        start=True, stop=True,
                                    )
                                    nc.scalar.activation(ed[:, kc, :], ps_sc[:], AF.Exp)
                                    nc.gpsimd.tensor_mul(ed[:, kc, :], ed[:, kc, :], md[:, kc, :])
                                    nc.tensor.matmul(ps_dd[:], ones2b[:], ed[:, kc, :],
                                                     start=(kc == 0), stop=(kc == 1))
                                rd05 = sp.tile([128, 256], F32, tag="rd05")
                                nc.vector.reciprocal(rd05[:], ps_dd[:])
                                if debug and l == 0 and b == 0 and h == 0:
                                    nc.sync.dma_start(out=dbg["dbg_ed"][:], in_=ed[:])
                                    nc.sync.dma_start(out=dbg["dbg_rd05"][:], in_=rd05[:])
                                for kc in range(2):
                                    nc.vector.tensor_mul(dah[:, kc, :], ed[:, kc, :], rd05[:])
                                    ps_sc2 = pssc.tile([128, 256], F32, tag="pssc")
                                    nc.tensor.matmul(
                                        ps_sc2[:],
                                        ks[hr:hr + 64, hp, q0 + kc * 128:q0 + (kc + 1) * 128],
                                        qs[hr:hr + 64, hp, q0:q0 + 256],
                                        start=True, stop=True,
                                    )
                                    nc.vector.tensor_add(dah[:, kc, :], ps_sc2[:], dah[:, kc, :])
                                    nc.scalar.activation(es[:, kc, :], dah[:, kc, :], AF.Exp)
                                    nc.gpsimd.tensor_mul(es[:, kc, :], es[:, kc, :], ms[:, kc, :])
                                # each PSUM accumulation group must run start..stop with no
                                # other start=True landing in the same bank in between
                                # (start clears has_written bank-wide)
                                for kc in range(2):
                                    nc.tensor.matmul(ps_rf[hr:hr + 64], ones1b[:], es[:, kc, :],
                                                     start=(kc == 0), stop=(kc == 1),
                                                     tile_position=(0, hr))
                                for kc in range(2):
                                    nc.tensor.matmul(
                                        ps_ctx[hr:hr + 64],
                                        vsg[:, lb * 2 + kc, h * 64:(h + 1) * 64],
                                        es[:, kc, :],
                                        start=(kc == 0), stop=(kc == 1),
                                        tile_position=(0, hr),
                                    )
                            rf2 = sp.tile([128, 256], F32, tag="rf2")
                            nc.vector.reciprocal(rf2[:], ps_rf)
                            if debug and l == 0 and b == 0 and hp == 0:
                                cnum_t = sp.tile([128, 256], F32, tag="cnum")
                                nc.scalar.activation(cnum_t[:], ps_ctx, AF.Copy)
                                nc.sync.dma_start(out=dbg["dbg_cnum"][:], in_=cnum_t[:])
                                den_t = sp.tile([128, 256], F32, tag="dent")
                                nc.scalar.activation(den_t[:], ps_rf, AF.Copy)
                                nc.sync.dma_start(out=dbg["dbg_den"][:], in_=den_t[:])
                                nc.sync.dma_start(out=dbg["dbg_rf2"][:], in_=rf2[:])
                            nc.vector.tensor_mul(ctxT[:, hp, :], ps_ctx, rf2[:])
                            if debug and l == 0 and b == 0 and hp == 0:
                                nc.sync.dma_start(out=dbg["dbg_es"][:], in_=es[:])
                        # out-projection + residual + LN1
                        z = zp.tile([128, DC, 256], F32R, tag="z")
                        for dc in range(DC):
                            ps_o = psacc.tile([128, 256], F32, tag="psacc")
                            for cc in range(DC):
                                nc.tensor.matmul(ps_o[:], wos[:, cc, dc * 128:(dc + 1) * 128],
                                                 ctxT[:, cc, :],
                                                 start=(cc == 0), stop=(cc == DC - 1))
                            tmpo = sp.tile([128, 256], F32, tag="tmpo")
                            nc.scalar.activation(tmpo[:], ps_o[:], AF.Identity, bias=bos[:, dc:dc + 1])
                            nc.vector.tensor_add(z[:, dc, :], tmpo[:],
                                                 xt[:, dc, ts0 + q0:ts0 + q0 + 256].bitcast(F32))
                        if debug and l == 0 and b == 0:
                            nc.sync.dma_start(out=dbg["dbg_ctx"][:], in_=ctxT.bitcast(F32))
                            nc.sync.dma_start(out=dbg["dbg_z"][:], in_=z.bitcast(F32))
                        z2 = zp.tile([128, DC, 256], F32R, tag="z2")
                        ps_mm = psmm.tile([128, 512], F32, tag="psmm")
                        ps_mean = ps_mm[:, 0:256]
                        ps_msq = ps_mm[:, 256:512]
                        for dc in range(DC):
                            nc.scalar.activation(z2[:, dc, :], z[:, dc, :], AF.Square)
                            nc.tensor.matmul(ps_mean, onesLN, z[:, dc, :],
                                             start=(dc == 0), stop=(dc == DC - 1))
                        for dc in range(DC):
                            nc.tensor.matmul(ps_msq, onesLN, z2[:, dc, :],
                                             start=(dc == 0), stop=(dc == DC - 1))
                        mean_sb = sp.tile([128, 256], F32, tag="mean")
                        nc.scalar.activation(mean_sb[:], ps_mean, AF.Copy)
                        var = sp.tile([128, 256], F32, tag="var")
                        nc.vector.tensor_mul(var[:], mean_sb[:], mean_sb[:])
                        nc.vector.tensor_sub(var[:], ps_msq, var[:])
                        sd = sp.tile([128, 256], F32, tag="sd")
                        nc.scalar.activation(sd[:], var[:], AF.Sqrt, bias=eps_t[:])
                        rstd = sp.tile([128, 256], F32, tag="rstd")
                        nc.vector.reciprocal(rstd[:], sd[:])
                        for dc in range(DC):
                            t1 = sp.tile([128, 256], F32, tag="t1")
                            nc.vector.tensor_sub(t1[:], z[:, dc, :].bitcast(F32), mean_sb[:])
                            nc.vector.tensor_mul(t1[:], t1[:], rstd[:])
                            nc.scalar.activation(y1t[:, dc, ts0 + q0:ts0 + q0 + 256], t1[:],
                                                 AF.Identity, bias=be1t[:, dc:dc + 1],
                                                 scale=g1t[:, dc:dc + 1])

            if debug and l == 0:
                nc.sync.dma_start(out=dbg["dbg_y1"][:], in_=y1t[:, :, 0:256].bitcast(F32))
            # ================= FFN phase =================
            with ExitStack() as fctx:
                wff = fctx.enter_context(tc.tile_pool(name=f"wff{l}", bufs=1))
                h1p = fctx.enter_context(tc.tile_pool(name=f"h1p{l}", bufs=1))
                z2p = fctx.enter_context(tc.tile_pool(name=f"z2p{l}", bufs=1))
                sp2 = fctx.enter_context(tc.tile_pool(name=f"sp2{l}", bufs=1))
                otp = fctx.enter_context(tc.tile_pool(name=f"otp{l}", bufs=1))
                psf = fctx.enter_context(tc.tile_pool(name=f"psf{l}", bufs=4, space="PSUM"))
                psf2 = fctx.enter_context(tc.tile_pool(name=f"psf2{l}", bufs=1, space="PSUM"))

                w1t = wff.tile([128, DC, FF], F32R, tag="w1t")
                nc.sync.dma_start(out=w1t[:], in_=wrearr(f"w1{l}"))
                w2t = wff.tile([128, FC, D], F32R, tag="w2t")
                nc.sync.dma_start(out=w2t[:], in_=wrearr(f"w2{l}"))

                last = l == L - 1
                if not last:
                    xt = xt_pool.tile([128, DC, T], F32R, tag="xt")

                for tt in range(4):
                    t0 = tt * 512
                    h1 = h1p.tile([128, FC, 512], F32R, tag="h1")
                    for fc in range(FC):
                        ps = psf.tile([128, 512], F32, tag="psf")
                        for kc in range(DC):
                            nc.tensor.matmul(ps[:], w1t[:, kc, fc * 128:(fc + 1) * 128],
                                             y1t[:, kc, t0:t0 + 512],
                                             start=(kc == 0), stop=(kc == DC - 1))
                        nc.scalar.activation(h1[:, fc, :], ps[:], AF.Gelu, bias=b1t[:, fc:fc + 1])
                    z2t = z2p.tile([128, DC, 512], F32R, tag="z2t")
                    for dc in range(DC):
                        ps = psf.tile([128, 512], F32, tag="psf")
                        for fc in range(FC):
                            nc.tensor.matmul(ps[:], w2t[:, fc, dc * 128:(dc + 1) * 128],
                                             h1[:, fc, :],
                                             start=(fc == 0), stop=(fc == FC - 1))
                        tmpf = sp2.tile([128, 512], F32, tag="tmpf")
                        nc.scalar.activation(tmpf[:], ps[:], AF.Identity, bias=b2t[:, dc:dc + 1])
                        nc.vector.tensor_add(z2t[:, dc, :], tmpf[:],
                                             y1t[:, dc, t0:t0 + 512].bitcast(F32))
                    # LN2
                    z2sq = z2p.tile([128, DC, 512], F32R, tag="z2sq")
                    ps_mean = psf2.tile([128, 512], F32, tag="psm1")
                    ps_msq = psf2.tile([128, 512], F32, tag="psm2")
                    for dc in range(DC):
                        nc.scalar.activation(z2sq[:, dc, :], z2t[:, dc, :], AF.Square)
                        nc.tensor.matmul(ps_mean[:], onesLN, z2t[:, dc, :],
                                         start=(dc == 0), stop=(dc == DC - 1))
                        nc.tensor.matmul(ps_msq[:], onesLN, z2sq[:, dc, :],
                                         start=(dc == 0), stop=(dc == DC - 1))
                    mean_sb = sp2.tile([128, 512], F32, tag="mean2")
                    nc.scalar.activation(mean_sb[:], ps_mean[:], AF.Copy)
                    var = sp2.tile([128, 512], F32, tag="var2")
                    nc.vector.tensor_mul(var[:], mean_sb[:], mean_sb[:])
                    nc.vector.tensor_sub(var[:], ps_msq[:], var[:])
                    sd = sp2.tile([128, 512], F32, tag="sd2")
                    nc.scalar.activation(sd[:], var[:], AF.Sqrt, bias=eps_t[:])
                    rstd = sp2.tile([128, 512], F32, tag="rstd2")
                    nc.vector.reciprocal(rstd[:], sd[:])
                    if last:
                        ot = otp.tile([128, DC, 512], F32, tag="ot")
                    for dc in range(DC):
                        t1 = sp2.tile([128, 512], F32, tag="t12")
                        nc.vector.tensor_sub(t1[:], z2t[:, dc, :].bitcast(F32), mean_sb[:])
                        nc.vector.tensor_mul(t1[:], t1[:], rstd[:])
                        dest = ot[:, dc, :] if last else xt[:, dc, t0:t0 + 512]
                        nc.scalar.activation(dest, t1[:], AF.Identity, bias=be2t[:, dc:dc + 1],
                                             scale=g2t[:, dc:dc + 1])
                    if last:
                        nc.sync.dma_start(out=outd_r[:, :, t0:t0 + 512], in_=ot[:])

    nc.compile()
    return nc


def _get_runner(debug=False):
    key = ("runner", debug)
    if key in _CACHE:
        return _CACHE[key]
    nc = _build_program(debug)

    import jax
    from jax.sharding import Mesh, PartitionSpec
    from jax.experimental.shard_map import shard_map
    from concourse import bass2jax

    bass2jax.install_neuronx_cc_hook()

    partition_name = nc.partition_id_tensor.name if nc.partition_id_tensor else None
    in_names, out_names, out_avals, zero_shapes = [], [], [], []
    for alloc in nc.m.functions[0].allocations:
        if not isinstance(alloc, mybir.MemoryLocationSet):
            continue
        name = alloc.memorylocations[0].name
        if alloc.kind == "ExternalInput":
            if name != partition_name:
                in_names.append(name)
        elif alloc.kind == "ExternalOutput":
            shape = tuple(alloc.tensor_shape)
            dtype = mybir.dt.np(alloc.dtype)
            out_names.append(name)
            out_avals.append(jax.core.ShapedArray(shape, dtype))
            zero_shapes.append((shape, dtype))
    n_params = len(in_names)
    n_outs = len(out_names)
    all_in_names = in_names + out_names
    if partition_name is not None:
        all_in_names = all_in_names + [partition_name]
    donate = tuple(range(n_params, n_params + n_outs))

    def _body(*args):
        operands = list(args)
        if partition_name is not None:
            operands.append(bass2jax.partition_id_tensor())
        outs = bass2jax._bass_exec_p.bind(
            *operands,
            out_avals=tuple(out_avals),
            in_names=tuple(all_in_names),
            out_names=tuple(out_names),
            lowering_input_output_aliases=(),
            sim_require_finite=True,
            sim_require_nnan=True,
            nc=nc,
        )
        return tuple(outs)

    devices = jax.devices()[:NCORES]
    mesh = Mesh(np.asarray(devices), ("core",))
    sharded = jax.jit(
        shard_map(_body, mesh=mesh,
                  in_specs=(PartitionSpec("core"),) * (n_params + n_outs),
                  out_specs=(PartitionSpec("core"),) * n_outs,
                  check_rep=False),
        donate_argnums=donate, keep_unused=True,
    )

    def run(in_maps):
        concat_in = [
            np.concatenate([np.asarray(m[name]) for m in in_maps], axis=0)
            for name in in_names
        ]
        concat_zeros = [
            np.zeros((NCORES * s[0], *s[1:]), dt) for s, dt in zero_shapes
        ]
        out_arrs = sharded(*concat_in, *concat_zeros)
        return [
            {name: np.asarray(out_arrs[i]).reshape(NCORES, *out_avals[i].shape)[c]
             for i, name in enumerate(out_names)}
            for c in range(NCORES)
        ]

    _CACHE[key] = run
    return run


def _prep_inputs(x, params, seq_attn_mask, dep_attn_mask):
    x = np.asarray(x, np.float32)
    seq_m = np.asarray(seq_attn_mask)
    dep_m = np.asarray(dep_attn_mask)

    consts = np.empty((128, 320), np.float32)
    consts[:, 0:128] = 2.0
    consts[:, 128:192] = 1.0
    consts[:, 192:320] = 1.0 / D

    shared = {"consts": consts}
    for l in range(L):
        pd = {k: np.asarray(v, np.float32) for k, v in params[l]["dep"].items()}
        ps_ = {k: np.asarray(v, np.float32) for k, v in params[l]["seq"].items()}
        shared[f"wq_d{l}"] = pd["wq"]
        shared[f"wk_d{l}"] = pd["wk"]
        shared[f"bq_d{l}"] = (pd["bq"] * 0.125).reshape(DC, 128)
        shared[f"bk_d{l}"] = pd["bk"].reshape(DC, 128)
        shared[f"wq_s{l}"] = ps_["wq"]
        shared[f"wk_s{l}"] = ps_["wk"]
        shared[f"wv_s{l}"] = ps_["wv"]
        shared[f"wo_s{l}"] = ps_["wo"]
        shared[f"bq_s{l}"] = (ps_["bq"] * 0.0625).reshape(DC, 128)
        shared[f"bk_s{l}"] = ps_["bk"].reshape(DC, 128)
        shared[f"bvrep{l}"] = np.broadcast_to(ps_["bv"], (128, D)).copy()
        shared[f"bo_s{l}"] = ps_["bo"].reshape(DC, 128)
        shared[f"w1{l}"] = ps_["w1"]
        shared[f"b1{l}"] = ps_["b1"].reshape(FC, 128)
        shared[f"w2{l}"] = ps_["w2"]
        shared[f"b2{l}"] = ps_["b2"].reshape(DC, 128)
        shared[f"g1{l}"] = ps_["ln1_g"].reshape(DC, 128)
        shared[f"be1{l}"] = ps_["ln1_b"].reshape(DC, 128)
        shared[f"g2{l}"] = ps_["ln2_g"].reshape(DC, 128)
        shared[f"be2{l}"] = ps_["ln2_b"].reshape(DC, 128)

    in_maps = []
    for c in range(NCORES):
        xc = x[c * BC:(c + 1) * BC].reshape(T, D)
        m = dict(shared)
        m["xT"] = np.ascontiguousarray(xc.T)
        # transposed multiplicative exp-masks: 1 where kept, 0 where masked
        mdep = (~dep_m[c * BC:(c + 1) * BC]).astype(np.float32)
        mseq = (~seq_m[c * BC:(c + 1) * BC]).astype(np.float32)
        m["m01_dep"] = np.ascontiguousarray(mdep.transpose(0, 2, 1)).reshape(BC, 2, 128, S).astype(ml_dtypes.bfloat16)
        m["m01_seq"] = np.ascontiguousarray(mseq.transpose(0, 2, 1)).reshape(BC, 2, 128, S).astype(ml_dtypes.bfloat16)
        in_maps.append(m)
    return in_maps


def kernel(x, params, seq_attn_mask, dep_attn_mask):
    run = _get_runner()
    in_maps = _prep_inputs(x, params, seq_attn_mask, dep_attn_mask)
    results = run(in_maps)
    out = np.empty((B, S, D), np.float32)
    for c in range(NCORES):
        out[c * BC:(c + 1) * BC] = results[c]["outT"].T.reshape(BC, S, D)
    return out


# revision 27
# speedup vs baseline: 5921.8710x; 5921.8710x over previous
"""Trainium2 Bass kernel for the dual-branch (dep/seq) transformer encoder.

Math notes (exact algebraic simplifications of the reference, not approximations):
- In the reference, `dep_out`/dep-FFN are dead code: only `dep_attn` (the dep
  branch's softmax probabilities) feeds forward into the seq branch scores.
- `hm = (help_scores > -1e8)` is always 1.0 since softmax output is in [0,1],
  so mixed scores = 0.5*dep_attn + 0.5*masked_seq_scores.
- Softmax is computed without max-subtraction (scores are O(1); masked entries
  reach exp() as exp(s)*mask01 with mask01 in {0,1}, which matches the
  reference's exp(-1e9 - max) == 0 exactly in fp32).

Layout: all activations are kept transposed ([feature, token]) so every linear
layer consumes the previous output directly as the stationary/moving matmul
operand and no on-chip transposes are needed. LayerNorm statistics and softmax
denominators (partition-dim reductions) are computed with all-ones matmuls on
the tensor engine, which also replicates them across partitions for free.

Sharding: data-parallel over batch; each of the 8 cores handles 8 batches.
"""

import numpy as np
import ml_dtypes
from contextlib import ExitStack

import concourse.bacc as bacc
import concourse.bass as bass
import concourse.tile as tile
from concourse import mybir

F32 = mybir.dt.float32
F32R = mybir.dt.float32r
BF16 = mybir.dt.bfloat16
AF = mybir.ActivationFunctionType

NCORES = 8
B, S, D = 64, 256, 512
H, DK = 8, 64
FF, L = 2048, 2
BC = B // NCORES          # batches per core
T = BC * S                # tokens per core
DC = D // 128             # feature chunks
FC = FF // 128            # ff chunks

_CACHE = {}


def _bcast_mid(ap2d, n):
    """[P, N] AP -> [P, n(step0), N] broadcast along a middle dim."""
    return bass.AP(tensor=ap2d.tensor, offset=ap2d.offset,
                   ap=[ap2d.ap[0], [0, n], ap2d.ap[1]])


def _build_program(debug=False):
    nc = bacc.Bacc("TRN2", target_bir_lowering=False)

    def din(name, shape, dt=F32R):
        return nc.dram_tensor(name, shape, dt, kind="ExternalInput")

    xTd = din("xT", [D, T])
    m01d = din("m01_dep", [BC, 2, 128, S], BF16)
    m01s = din("m01_seq", [BC, 2, 128, S], BF16)
    constsd = din("consts", [128, 320])
    wd = {}
    for l in range(L):
        for nm, shape in [
            ("wq_d", [D, D]), ("wk_d", [D, D]),
            ("wq_s", [D, D]), ("wk_s", [D, D]), ("wv_s", [D, D]), ("wo_s", [D, D]),
            ("w1", [D, FF]), ("w2", [FF, D]),
        ]:
            wd[f"{nm}{l}"] = din(f"{nm}{l}", shape)
        for nm in ["bq_d", "bk_d", "bq_s", "bk_s", "bo_s", "b2", "g1", "be1", "g2", "be2"]:
            wd[f"{nm}{l}"] = din(f"{nm}{l}", [DC, 128], F32)
        wd[f"b1{l}"] = din(f"b1{l}", [FC, 128], F32)
        wd[f"bvrep{l}"] = din(f"bvrep{l}", [128, D], F32)
    outd = nc.dram_tensor("outT", [D, T], F32, kind="ExternalOutput")
    outd_r = outd.rearrange("(c p) t -> p c t", p=128)
    dbg = {}
    if debug:
        for nm, shape, dt in [
            ("dbg_qd", [128, DC, 256], BF16), ("dbg_kd", [128, DC, 256], BF16),
            ("dbg_vs", [128, 4, D], BF16), ("dbg_ed", [128, 2, 256], BF16),
            ("dbg_rd05", [128, 256], F32), ("dbg_es", [128, 2, 256], BF16),
            ("dbg_ctx", [128, DC, 256], F32), ("dbg_z", [128, DC, 256], F32),
            ("dbg_y1", [128, DC, 256], F32),
            ("dbg_cnum", [128, 256], F32), ("dbg_rf2", [128, 256], F32),
            ("dbg_den", [128, 256], F32), ("dbg_qdfull", [128, DC, 512], BF16),
            ("dbg_xt", [128, DC, 512], F32), ("dbg_vps", [128, 512], F32),
        ]:
            dbg[nm] = nc.dram_tensor(nm, shape, dt, kind="ExternalOutput")

    def wrearr(name):
        return wd[name].rearrange("(c p) o -> p c o", p=128)

    def brearr(name):
        return wd[name].rearrange("c p -> p c")

    with tile.TileContext(nc) as tc, ExitStack() as ctx:
        const = ctx.enter_context(tc.tile_pool(name="const", bufs=1))
        biasp = ctx.enter_context(tc.tile_pool(name="bias", bufs=1))
        xt_pool = ctx.enter_context(tc.tile_pool(name="xtp", bufs=1))
        y1t_pool = ctx.enter_context(tc.tile_pool(name="y1tp", bufs=1))

        consts = const.tile([128, 320], F32R, tag="consts")
        nc.sync.dma_start(out=consts[:], in_=constsd[:])
        ones2 = consts[:, 0:128]     # value 2.0   (dep denominator, folds the 0.5)
        ones1 = consts[:, 128:192]   # value 1.0   (seq denominator, M=64)
        onesLN = consts[:, 192:320]  # value 1/512 (layernorm stats)
        eps_t = const.tile([128, 1], F32, tag="eps")
        nc.vector.memset(eps_t[:], 1e-5)
        ones2b = const.tile([128, 128], BF16, tag="ones2b")
        nc.vector.tensor_copy(ones2b[:], ones2)
        ones1b = const.tile([128, 64], BF16, tag="ones1b")
        nc.vector.tensor_copy(ones1b[:], ones1)

        xt = xt_pool.tile([128, DC, T], F32R, tag="xt")
        nc.sync.dma_start(out=xt[:], in_=xTd.rearrange("(c p) t -> p c t", p=128))

        for l in range(L):
            # ---------------- biases for this layer ----------------
            bqd = biasp.tile([128, DC], F32, tag="bqd")
            nc.sync.dma_start(out=bqd[:], in_=brearr(f"bq_d{l}"))
            bkd = biasp.tile([128, DC], F32, tag="bkd")
            nc.sync.dma_start(out=bkd[:], in_=brearr(f"bk_d{l}"))
            bqs = biasp.tile([128, DC], F32, tag="bqs")
            nc.sync.dma_start(out=bqs[:], in_=brearr(f"bq_s{l}"))
            bks = biasp.tile([128, DC], F32, tag="bks")
            nc.sync.dma_start(out=bks[:], in_=brearr(f"bk_s{l}"))
            bos = biasp.tile([128, DC], F32, tag="bos")
            nc.sync.dma_start(out=bos[:], in_=brearr(f"bo_s{l}"))
            b2t = biasp.tile([128, DC], F32, tag="b2t")
            nc.sync.dma_start(out=b2t[:], in_=brearr(f"b2{l}"))
            g1t = biasp.tile([128, DC], F32, tag="g1t")
            nc.sync.dma_start(out=g1t[:], in_=brearr(f"g1{l}"))
            be1t = biasp.tile([128, DC], F32, tag="be1t")
            nc.sync.dma_start(out=be1t[:], in_=brearr(f"be1{l}"))
            g2t = biasp.tile([128, DC], F32, tag="g2t")
            nc.sync.dma_start(out=g2t[:], in_=brearr(f"g2{l}"))
            be2t = biasp.tile([128, DC], F32, tag="be2t")
            nc.sync.dma_start(out=be2t[:], in_=brearr(f"be2{l}"))
            b1t = biasp.tile([128, FC], F32, tag="b1t")
            nc.sync.dma_start(out=b1t[:], in_=wd[f"b1{l}"].rearrange("c p -> p c"))
            bvr = biasp.tile([128, D], F32, tag="bvr")
            nc.sync.dma_start(out=bvr[:], in_=wd[f"bvrep{l}"][:])

            y1t = y1t_pool.tile([128, DC, T], F32R, tag="y1t")

            # ================= attention phase =================
            with ExitStack() as actx:
                wat = actx.enter_context(tc.tile_pool(name=f"wat{l}", bufs=1))
                qkp = actx.enter_context(tc.tile_pool(name=f"qk{l}", bufs=2))
                vsp = actx.enter_context(tc.tile_pool(name=f"vs{l}", bufs=2))
                mp = actx.enter_context(tc.tile_pool(name=f"mp{l}", bufs=2))
                ep = actx.enter_context(tc.tile_pool(name=f"ep{l}", bufs=4))
                sp = actx.enter_context(tc.tile_pool(name=f"sp{l}", bufs=1))
                cp = actx.enter_context(tc.tile_pool(name=f"cp{l}", bufs=1))
                zp = actx.enter_context(tc.tile_pool(name=f"zp{l}", bufs=2))
                psb = actx.enter_context(tc.tile_pool(name=f"psb{l}", bufs=1, space="PSUM"))
                pssc = actx.enter_context(tc.tile_pool(name=f"pssc{l}", bufs=3, space="PSUM"))
                psacc = actx.enter_context(tc.tile_pool(name=f"psacc{l}", bufs=2, space="PSUM"))
                psrc = actx.enter_context(tc.tile_pool(name=f"psrc{l}", bufs=1, space="PSUM"))
                psmm = actx.enter_context(tc.tile_pool(name=f"psmm{l}", bufs=1, space="PSUM"))

                wqd = wat.tile([128, DC, D], F32R, tag="wqd")
                nc.sync.dma_start(out=wqd[:], in_=wrearr(f"wq_d{l}"))
                wkd = wat.tile([128, DC, D], F32R, tag="wkd")
                nc.sync.dma_start(out=wkd[:], in_=wrearr(f"wk_d{l}"))
                wqs = wat.tile([128, DC, D], F32R, tag="wqs")
                nc.sync.dma_start(out=wqs[:], in_=wrearr(f"wq_s{l}"))
                wks = wat.tile([128, DC, D], F32R, tag="wks")
                nc.sync.dma_start(out=wks[:], in_=wrearr(f"wk_s{l}"))
                wvs = wat.tile([128, DC, D], F32R, tag="wvs")
                nc.sync.dma_start(out=wvs[:], in_=wrearr(f"wv_s{l}"))
                wos = wat.tile([128, DC, D], F32R, tag="wos")
                nc.sync.dma_start(out=wos[:], in_=wrearr(f"wo_s{l}"))

                for g in range(4):  # groups of 2 batches (512 tokens)
                    ts0 = g * 512
                    qd = qkp.tile([128, DC, 512], BF16, tag="qd")
                    kd = qkp.tile([128, DC, 512], BF16, tag="kd")
                    qs = qkp.tile([128, DC, 512], BF16, tag="qs")
                    ks = qkp.tile([128, DC, 512], BF16, tag="ks")
                    vsg = vsp.tile([128, 4, D], BF16, tag="vs")
                    for w, bias, dest, scale in [
                        (wqd, bqd, qd, 0.125),
                        (wkd, bkd, kd, 1.0),
                        (wqs, bqs, qs, 0.0625),
                        (wks, bks, ks, 1.0),
                    ]:
                        for dc in range(DC):
                            ps = psb.tile([128, 512], F32, tag="psb")
                            for kc in range(DC):
                                nc.tensor.matmul(
                                    ps[:], w[:, kc, dc * 128:(dc + 1) * 128],
                                    xt[:, kc, ts0:ts0 + 512],
                                    start=(kc == 0), stop=(kc == DC - 1),
                                )
                            nc.scalar.activation(dest[:, dc, :], ps[:], AF.Identity,
                                                 bias=bias[:, dc:dc + 1], scale=scale)
                    for tti in range(4):  # V in [token, feature] layout
                        ps = psb.tile([128, 512], F32, tag="psb")
                        for kc in range(DC):
                            nc.tensor.matmul(
                                ps[:], xt[:, kc, ts0 + tti * 128:ts0 + (tti + 1) * 128],
                                wvs[:, kc, :],
                                start=(kc == 0), stop=(kc == DC - 1),
                            )
                        nc.vector.tensor_add(vsg[:, tti, :], ps[:], bvr[:])
                        if debug and g == 0 and tti == 0:
                            vps_t = sp.tile([128, 512], F32, tag="vps")
                            nc.scalar.activation(vps_t[:], ps[:], AF.Copy)
                            nc.sync.dma_start(out=dbg["dbg_vps"][:], in_=vps_t[:])

                    if debug and g == 0:
                        nc.sync.dma_start(out=dbg["dbg_qdfull"][:], in_=qd[:])
                        nc.sync.dma_start(out=dbg["dbg_xt"][:], in_=xt[:, :, 0:512].bitcast(F32))
                        nc.sync.dma_start(out=dbg["dbg_qd"][:], in_=qd[:, :, 0:256])
                        nc.sync.dma_start(out=dbg["dbg_kd"][:], in_=kd[:, :, 0:256])
                        nc.sync.dma_start(out=dbg["dbg_vs"][:], in_=vsg[:])
                    for lb in range(2):
                        b = g * 2 + lb
                        q0 = lb * 256
                        md = mp.tile([128, 2, S], BF16, tag="md")
                        nc.sync.dma_start(out=md[:], in_=m01d[b].rearrange("c p q -> p c q"))
                        ms = mp.tile([128, 2, S], BF16, tag="ms")
                        nc.sync.dma_start(out=ms[:], in_=m01s[b].rearrange("c p q -> p c q"))
                        ctxT = cp.tile([128, DC, 256], F32R, tag="ctx")
                        for hp in range(4):  # head pairs; pair hp = chunk hp, heads at rows 0/64
                            ps_rc = psrc.tile([128, 512], F32, tag="psrc")
                            ps_rf = ps_rc[:, 0:256]
                            ps_ctx = ps_rc[:, 256:512]
                            for hh in range(2):
                                h = hp * 2 + hh
                                hr = hh * 64
                                ed = ep.tile([128, 2, 256], BF16, tag="ed")
                                dah = ep.tile([128, 2, 256], F32, tag="dah")
                                es = ep.tile([128, 2, 256], BF16, tag="es")
                                ps_dd = psacc.tile([128, 256], F32, tag="psacc")
                                ed_w = ed.rearrange("p a b -> p (a b)")
                                md_w = md.rearrange("p a b -> p (a b)")
                                ps_sc = pssc.tile([128, 512], F32, tag="pssc")
                                for kc in range(2):
                                    nc.tensor.matmul(
                                        ps_sc[:, kc * 256:(kc + 1) * 256],
                                        kd[hr:hr + 64, hp, q0 + kc * 128:q0 + (kc + 1) * 128],
                                        qd[hr:hr + 64, hp, q0:q0 + 256],
                                        start=True, stop=True,
                                    )
                                nc.scalar.activation(ed_w, ps_sc[:], AF.Exp)
                                nc.gpsimd.tensor_mul(ed_w, ed_w, md_w)
                                for kc in range(2):
                                    nc.tensor.matmul(ps_dd[:], ones2b[:], ed[:, kc, :],
                                                     start=(kc == 0), stop=(kc == 1))
                                rd05 = sp.tile([128, 256], F32, tag="rd05")
                                nc.vector.reciprocal(rd05[:], ps_dd[:])
                                if debug and l == 0 and b == 0 and h == 0:
                                    nc.sync.dma_start(out=dbg["dbg_ed"][:], in_=ed[:])
                                    nc.sync.dma_start(out=dbg["dbg_rd05"][:], in_=rd05[:])
                                ps_sc2 = pssc.tile([128, 512], F32, tag="pssc")
                                nc.vector.tensor_mul(
                                    dah[:], ed[:], _bcast_mid(rd05[:], 2).rearrange("p a b -> p a b"))
                                for kc in range(2):
                                    nc.tensor.matmul(
                                        ps_sc2[:, kc * 256:(kc + 1) * 256],
                                        ks[hr:hr + 64, hp, q0 + kc * 128:q0 + (kc + 1) * 128],
                                        qs[hr:hr + 64, hp, q0:q0 + 256],
                                        start=True, stop=True,
                                    )
                                dah_w = dah.rearrange("p a b -> p (a b)")
                                es_w = es.rearrange("p a b -> p (a b)")
                                ms_w = ms.rearrange("p a b -> p (a b)")
                                nc.vector.tensor_add(dah_w, ps_sc2[:], dah_w)
                                nc.scalar.activation(es_w, dah_w, AF.Exp)
                                nc.gpsimd.tensor_mul(es_w, es_w, ms_w)
                                # each PSUM accumulation group must run start..stop with no
                                # other start=True landing in the same bank in between
                                # (start clears has_written bank-wide)
                                for kc in range(2):
                                    nc.tensor.matmul(ps_rf[hr:hr + 64], ones1b[:], es[:, kc, :],
                                                     start=(kc == 0), stop=(kc == 1),
                                                     tile_position=(0, hr))
                                for kc in range(2):
                                    nc.tensor.matmul(
                                        ps_ctx[hr:hr + 64],
                                        vsg[:, lb * 2 + kc, h * 64:(h + 1) * 64],
                                        es[:, kc, :],
                                        start=(kc == 0), stop=(kc == 1),
                                        tile_position=(0, hr),
                                    )
                            rf2 = sp.tile([128, 256], F32, tag="rf2")
                            nc.vector.reciprocal(rf2[:], ps_rf)
                            if debug and l == 0 and b == 0 and hp == 0:
                                cnum_t = sp.tile([128, 256], F32, tag="cnum")
                                nc.scalar.activation(cnum_t[:], ps_ctx, AF.Copy)
                                nc.sync.dma_start(out=dbg["dbg_cnum"][:], in_=cnum_t[:])
                                den_t = sp.tile([128, 256], F32, tag="dent")
                                nc.scalar.activation(den_t[:], ps_rf, AF.Copy)
                                nc.sync.dma_start(out=dbg["dbg_den"][:], in_=den_t[:])
                                nc.sync.dma_start(out=dbg["dbg_rf2"][:], in_=rf2[:])
                            nc.vector.tensor_mul(ctxT[:, hp, :], ps_ctx, rf2[:])
                            if debug and l == 0 and b == 0 and hp == 0:
                                nc.sync.dma_start(out=dbg["dbg_es"][:], in_=es[:])
                        # out-projection + residual + LN1
                        z = zp.tile([128, DC, 256], F32R, tag="z")
                        for dc in range(DC):
                            ps_o = psacc.tile([128, 256], F32, tag="psacc")
                            for cc in range(DC):
                                nc.tensor.matmul(ps_o[:], wos[:, cc, dc * 128:(dc + 1) * 128],
                                                 ctxT[:, cc, :],
                                                 start=(cc == 0), stop=(cc == DC - 1))
                            nc.vector.scalar_tensor_tensor(
                                z[:, dc, :], ps_o[:], bos[:, dc:dc + 1],
                                xt[:, dc, ts0 + q0:ts0 + q0 + 256].bitcast(F32),
                                mybir.AluOpType.add, mybir.AluOpType.add)
                        if debug and l == 0 and b == 0:
                            nc.sync.dma_start(out=dbg["dbg_ctx"][:], in_=ctxT.bitcast(F32))
                            nc.sync.dma_start(out=dbg["dbg_z"][:], in_=z.bitcast(F32))
                        z2 = zp.tile([128, DC, 256], F32R, tag="z2")
                        ps_mm = psmm.tile([128, 512], F32, tag="psmm")
                        ps_mean = ps_mm[:, 0:256]
                        ps_msq = ps_mm[:, 256:512]
                        nc.scalar.activation(z2.rearrange("p a b -> p (a b)"),
                                             z.rearrange("p a b -> p (a b)"), AF.Square)
                        for dc in range(DC):
                            nc.tensor.matmul(ps_mean, onesLN, z[:, dc, :],
                                             start=(dc == 0), stop=(dc == DC - 1))
                        for dc in range(DC):
                            nc.tensor.matmul(ps_msq, onesLN, z2[:, dc, :],
                                             start=(dc == 0), stop=(dc == DC - 1))
                        mean_sb = sp.tile([128, 256], F32, tag="mean")
                        nc.scalar.activation(mean_sb[:], ps_mean, AF.Copy)
                        var = sp.tile([128, 256], F32, tag="var")
                        nc.vector.tensor_mul(var[:], mean_sb[:], mean_sb[:])
                        nc.vector.tensor_sub(var[:], ps_msq, var[:])
                        sd = sp.tile([128, 256], F32, tag="sd")
                        nc.scalar.activation(sd[:], var[:], AF.Sqrt, bias=eps_t[:])
                        rstd = sp.tile([128, 256], F32, tag="rstd")
                        nc.vector.reciprocal(rstd[:], sd[:])
                        t1 = sp.tile([128, DC, 256], F32, tag="t1")
                        nc.vector.tensor_sub(t1[:], z.bitcast(F32)[:], _bcast_mid(mean_sb[:], DC))
                        nc.vector.tensor_mul(t1[:], t1[:], _bcast_mid(rstd[:], DC))
                        for dc in range(DC):
                            nc.scalar.activation(y1t[:, dc, ts0 + q0:ts0 + q0 + 256], t1[:, dc, :],
                                                 AF.Identity, bias=be1t[:, dc:dc + 1],
                                                 scale=g1t[:, dc:dc + 1])

            if debug and l == 0:
                nc.sync.dma_start(out=dbg["dbg_y1"][:], in_=y1t[:, :, 0:256].bitcast(F32))
            # ================= FFN phase =================
            with ExitStack() as fctx:
                wff = fctx.enter_context(tc.tile_pool(name=f"wff{l}", bufs=1))
                h1p = fctx.enter_context(tc.tile_pool(name=f"h1p{l}", bufs=1))
                z2p = fctx.enter_context(tc.tile_pool(name=f"z2p{l}", bufs=1))
                sp2 = fctx.enter_context(tc.tile_pool(name=f"sp2{l}", bufs=1))
                otp = fctx.enter_context(tc.tile_pool(name=f"otp{l}", bufs=1))
                psf = fctx.enter_context(tc.tile_pool(name=f"psf{l}", bufs=4, space="PSUM"))
                psf2 = fctx.enter_context(tc.tile_pool(name=f"psf2{l}", bufs=1, space="PSUM"))

                w1t = wff.tile([128, DC, FF], F32R, tag="w1t")
                nc.sync.dma_start(out=w1t[:], in_=wrearr(f"w1{l}"))
                w2t = wff.tile([128, FC, D], F32R, tag="w2t")
                nc.sync.dma_start(out=w2t[:], in_=wrearr(f"w2{l}"))

                last = l == L - 1
                if not last:
                    xt = xt_pool.tile([128, DC, T], F32R, tag="xt")

                for tt in range(4):
                    t0 = tt * 512
                    h1 = h1p.tile([128, FC, 512], F32R, tag="h1")
                    for fc in range(FC):
                        ps = psf.tile([128, 512], F32, tag="psf")
                        for kc in range(DC):
                            nc.tensor.matmul(ps[:], w1t[:, kc, fc * 128:(fc + 1) * 128],
                                             y1t[:, kc, t0:t0 + 512],
                                             start=(kc == 0), stop=(kc == DC - 1))
                        nc.scalar.activation(h1[:, fc, :], ps[:], AF.Gelu, bias=b1t[:, fc:fc + 1])
                    z2t = z2p.tile([128, DC, 512], F32R, tag="z2t")
                    for dc in range(DC):
                        ps = psf.tile([128, 512], F32, tag="psf")
                        for fc in range(FC):
                            nc.tensor.matmul(ps[:], w2t[:, fc, dc * 128:(dc + 1) * 128],
                                             h1[:, fc, :],
                                             start=(fc == 0), stop=(fc == FC - 1))
                        nc.vector.scalar_tensor_tensor(
                            z2t[:, dc, :], ps[:], b2t[:, dc:dc + 1],
                            y1t[:, dc, t0:t0 + 512].bitcast(F32),
                            mybir.AluOpType.add, mybir.AluOpType.add)
                    # LN2
                    z2sq = z2p.tile([128, DC, 512], F32R, tag="z2sq")
                    ps_mean = psf2.tile([128, 512], F32, tag="psm1")
                    ps_msq = psf2.tile([128, 512], F32, tag="psm2")
                    nc.scalar.activation(z2sq.rearrange("p a b -> p (a b)"),
                                         z2t.rearrange("p a b -> p (a b)"), AF.Square)
                    for dc in range(DC):
                        nc.tensor.matmul(ps_mean[:], onesLN, z2t[:, dc, :],
                                         start=(dc == 0), stop=(dc == DC - 1))
                    for dc in range(DC):
                        nc.tensor.matmul(ps_msq[:], onesLN, z2sq[:, dc, :],
                                         start=(dc == 0), stop=(dc == DC - 1))
                    mean_sb = sp2.tile([128, 512], F32, tag="mean2")
                    nc.scalar.activation(mean_sb[:], ps_mean[:], AF.Copy)
                    var = sp2.tile([128, 512], F32, tag="var2")
                    nc.vector.tensor_mul(var[:], mean_sb[:], mean_sb[:])
                    nc.vector.tensor_sub(var[:], ps_msq[:], var[:])
                    sd = sp2.tile([128, 512], F32, tag="sd2")
                    nc.scalar.activation(sd[:], var[:], AF.Sqrt, bias=eps_t[:])
                    rstd = sp2.tile([128, 512], F32, tag="rstd2")
                    nc.vector.reciprocal(rstd[:], sd[:])
                    if last:
                        ot = otp.tile([128, DC, 512], F32, tag="ot")
                    t1 = sp2.tile([128, DC, 512], F32, tag="t12")
                    nc.vector.tensor_sub(t1[:], z2t.bitcast(F32)[:], _bcast_mid(mean_sb[:], DC))
                    nc.vector.tensor_mul(t1[:], t1[:], _bcast_mid(rstd[:], DC))
                    for dc in range(DC):
                        dest = ot[:, dc, :] if last else xt[:, dc, t0:t0 + 512]
                        nc.scalar.activation(dest, t1[:, dc, :], AF.Identity, bias=be2t[:, dc:dc + 1],
                                             scale=g2t[:, dc:dc + 1])
                    if last:
                        nc.sync.dma_start(out=outd_r[:, :, t0:t0 + 512], in_=ot[:])

    nc.compile()
    return nc


def _get_runner(debug=False):
    key = ("runner", debug)
    if key in _CACHE:
        return _CACHE[key]
    nc = _build_program(debug)

    import jax
    from jax.sharding import Mesh, PartitionSpec
    from jax.experimental.shard_map import shard_map
    from concourse import bass2jax

    bass2jax.install_neuronx_cc_hook()

    partition_name = nc.partition_id_tensor.name if nc.partition_id_tensor else None
    in_names, out_names, out_avals, zero_shapes = [], [], [], []
    for alloc in nc.m.functions[0].allocations:
        if not isinstance(alloc, mybir.MemoryLocationSet):
            continue
        name = alloc.memorylocations[0].name
        if alloc.kind == "ExternalInput":
            if name != partition_name:
                in_names.append(name)
        elif alloc.kind == "ExternalOutput":
            shape = tuple(alloc.tensor_shape)
            dtype = mybir.dt.np(alloc.dtype)
            out_names.append(name)
            out_avals.append(jax.core.ShapedArray(shape, dtype))
            zero_shapes.append((shape, dtype))
    n_params = len(in_names)
    n_outs = len(out_names)
    all_in_names = in_names + out_names
    if partition_name is not None:
        all_in_names = all_in_names + [partition_name]
    donate = tuple(range(n_params, n_params + n_outs))

    def _body(*args):
        operands = list(args)
        if partition_name is not None:
            operands.append(bass2jax.partition_id_tensor())
        outs = bass2jax._bass_exec_p.bind(
            *operands,
            out_avals=tuple(out_avals),
            in_names=tuple(all_in_names),
            out_names=tuple(out_names),
            lowering_input_output_aliases=(),
            sim_require_finite=True,
            sim_require_nnan=True,
            nc=nc,
        )
        return tuple(outs)

    devices = jax.devices()[:NCORES]
    mesh = Mesh(np.asarray(devices), ("core",))
    sharded = jax.jit(
        shard_map(_body, mesh=mesh,
                  in_specs=(PartitionSpec("core"),) * (n_params + n_outs),
                  out_specs=(PartitionSpec("core"),) * n_outs,
                  check_rep=False),
        donate_argnums=donate, keep_unused=True,
    )

    def run(in_maps):
        concat_in = [
            np.concatenate([np.asarray(m[name]) for m in in_maps], axis=0)
            for name in in_names
        ]
        concat_zeros = [
            np.zeros((NCORES * s[0], *s[1:]), dt) for s, dt in zero_shapes
        ]
        out_arrs = sharded(*concat_in, *concat_zeros)
        return [
            {name: np.asarray(out_arrs[i]).reshape(NCORES, *out_avals[i].shape)[c]
             for i, name in enumerate(out_names)}
            for c in range(NCORES)
        ]

    _CACHE[key] = run
    return run


def _prep_inputs(x, params, seq_attn_mask, dep_attn_mask):
    x = np.asarray(x, np.float32)
    seq_m = np.asarray(seq_attn_mask)
    dep_m = np.asarray(dep_attn_mask)

    consts = np.empty((128, 320), np.float32)
    consts[:, 0:128] = 2.0
    consts[:, 128:192] = 1.0
    consts[:, 192:320] = 1.0 / D

    shared = {"consts": consts}
    for l in range(L):
        pd = {k: np.asarray(v, np.float32) for k, v in params[l]["dep"].items()}
        ps_ = {k: np.asarray(v, np.float32) for k, v in params[l]["seq"].items()}
        shared[f"wq_d{l}"] = pd["wq"]
        shared[f"wk_d{l}"] = pd["wk"]
        shared[f"bq_d{l}"] = (pd["bq"] * 0.125).reshape(DC, 128)
        shared[f"bk_d{l}"] = pd["bk"].reshape(DC, 128)
        shared[f"wq_s{l}"] = ps_["wq"]
        shared[f"wk_s{l}"] = ps_["wk"]
        shared[f"wv_s{l}"] = ps_["wv"]
        shared[f"wo_s{l}"] = ps_["wo"]
        shared[f"bq_s{l}"] = (ps_["bq"] * 0.0625).reshape(DC, 128)
        shared[f"bk_s{l}"] = ps_["bk"].reshape(DC, 128)
        shared[f"bvrep{l}"] = np.broadcast_to(ps_["bv"], (128, D)).copy()
        shared[f"bo_s{l}"] = ps_["bo"].reshape(DC, 128)
        shared[f"w1{l}"] = ps_["w1"]
        shared[f"b1{l}"] = ps_["b1"].reshape(FC, 128)
        shared[f"w2{l}"] = ps_["w2"]
        shared[f"b2{l}"] = ps_["b2"].reshape(DC, 128)
        shared[f"g1{l}"] = ps_["ln1_g"].reshape(DC, 128)
        shared[f"be1{l}"] = ps_["ln1_b"].reshape(DC, 128)
        shared[f"g2{l}"] = ps_["ln2_g"].reshape(DC, 128)
        shared[f"be2{l}"] = ps_["ln2_b"].reshape(DC, 128)

    in_maps = []
    for c in range(NCORES):
        xc = x[c * BC:(c + 1) * BC].reshape(T, D)
        m = dict(shared)
        m["xT"] = np.ascontiguousarray(xc.T)
        # transposed multiplicative exp-masks: 1 where kept, 0 where masked
        mdep = (~dep_m[c * BC:(c + 1) * BC]).astype(np.float32)
        mseq = (~seq_m[c * BC:(c + 1) * BC]).astype(np.float32)
        m["m01_dep"] = np.ascontiguousarray(mdep.transpose(0, 2, 1)).reshape(BC, 2, 128, S).astype(ml_dtypes.bfloat16)
        m["m01_seq"] = np.ascontiguousarray(mseq.transpose(0, 2, 1)).reshape(BC, 2, 128, S).astype(ml_dtypes.bfloat16)
        in_maps.append(m)
    return in_maps


def kernel(x, params, seq_attn_mask, dep_attn_mask):
    run = _get_runner()
    in_maps = _prep_inputs(x, params, seq_attn_mask, dep_attn_mask)
    results = run(in_maps)
    out = np.empty((B, S, D), np.float32)
    for c in range(NCORES):
        out[c * BC:(c + 1) * BC] = results[c]["outT"].T.reshape(BC, S, D)
    return out
